# revision 53
# baseline (speedup 1.0000x reference)
"""CondConv2d (MoE-routed conv) Trainium2 kernel — split-fp8 DoubleRow edition.

Reference computation (per sample b):
    pooled  = mean(x[b], HW)                          [C]
    r       = sigmoid(pooled @ Wr^T + br)             [E]
    w_b     = (r @ weight).reshape(O, C, 3, 3)
    bias_b  = r @ bias                                [O]
    out[b]  = conv2d(x[b], w_b, pad=1) + bias_b

Sharding: data-parallel over batch, 4 samples per core on 8 cores; the
small expert weight bank is replicated to every core (no collectives).

Numerics: the conv runs on the PE in fp8e4 (e4m3) DoubleRow perf mode
(0.5 cycles/row — 2x bf16).  Pure e4m3 misses the 2e-2 gate (measured
3.5e-2), so both operands are hi/lo split:
    x*SX = xh + xl,  w*SW = wh + wl   (each part e4m3, lo ~2^-4 of hi)
    out  = (wh+wl)@xh + wh@xl [+ wl@xl on one tap]  =  w@x - wl@xl
The dropped wl@xl terms contribute ~6e-4.  SW=2^10/SX=2^5 keep all
parts inside IEEE e4m3's +-240 range; the 2^-15 descale is folded into
the ACT drain (out = psum*2^-15 + bias_b, written bf16, upcast on host).
Routing pools only the xh half (the xl contribution to the mean is
~4e-4 absolute, ~0.3% on the output after the sigmoid) — measured
end-to-end rel err 2.5e-3 vs the 2e-2 gate.

Per-core dataflow (one Tile program):
  - x arrives from host pre-split/packed per sample as one fp8 tile
    [C, 1+3364+1+3364+1] = [guard | xh(58x58 padded) | guard | xl | guard];
    the zero row/col padding makes every conv tap a CONTIGUOUS 464-element
    window (8 output rows x 58 cols, pad cols wrap), so DoubleRow rhs APs
    stay exactly 3-D [128, 2, 464] as the ISA requires.
  - conv per (oc, row-block): 14 DoubleRow matmuls, K=256 each:
      9x DR-A  lhsT=[wh_t, wl_t]      rhs=[xh_t, xh_t]   -> w@xh
      4x DR-B  lhsT=[wh_2k, wh_2k+1]  rhs=[xl_2k, xl_2k+1] -> wh@xl pairs
      1x DR-C  lhsT=[wh_8, wl_8]      rhs=[xl_8, xl_8]    -> w@xl (tap 8)
    PSUM [128, 464] fp32, one bank per block, 7 blocks in flight.
  - weight-gen per (sample, oc): 8 DVE tensor_scalar muls (fp16, 4x DVE
    mode) + 3 in-place DVE tree adds (2x mode) form w_s = SW*sum_e r_e W_e;
    GPSIMD casts wh=fp8(w_s) and subtracts wl=fp8(w_s-wh) back-to-back on
    its own queue so the weight path never sits behind ACT drains.  The
    FMA-chain form (scalar_tensor_tensor) supports NO fast DVE modes; the
    mul/add tree does.
  - engine roles: ACT = xh pooling (accum), sigmoid, PSUM drains (descale
    + bias + bf16); DVE = weight mul/add trees + routing smalls; GPSIMD =
    partition_all_reduce + wh/wl split; PE = conv only.

Scheduling (this is where the previous revision lost 33us): all engine
queues are IN-ORDER, so emission order is execution order per engine.
  - a fused [128,32] consts DMA replaces 3 tiny ones (each dma_start
    holds SP.SEQ ~650ns and HWDGE ~630ns).
  - the expert bank loads in 8 tap-range chunks (4 per oc) interleaved
    with the x prefetch DMAs so sample 0/1's startup chain and the bank
    never head-of-line block each other on the serial DMA device.
  - weight-gen for (0,0)/(0,1)/(1,0) is chunked to chase the bank DMAs;
    conv groups (0,0)/(0,1) issue tap-major following those chunks; all
    later groups run block-major with whole-tile weight-gen.
  - a dummy Sigmoid is the first ACT instruction, hoisting the 1.28us
    activation-table load off sample 0's critical path.
  - per-sample pipelining: routing(b+1) pools emit before conv(b,0)'s
    drains on ACT; weight-gen(b+1) emits between the two conv groups.
  - PE warm-up matmuls absorb the pstate ramp during the ~8us startup
    (the cost model re-throttles the PE after idle gaps, so every stall
    also costs 2-4x on the next ~3us of matmuls).
  - output DMAs split in halves (quarters for the final group) to cut
    head-of-line blocking and the end-of-program tail.

Cost-model accounting (instruction_cost_v2): conv = 56 groups x 14 DR x
464 x 0.5 cyc = 181.9k PE-cycles = 75.8us @2.4GHz (vs 94us bf16 9-tap);
DMA ~41us serialized (x fp8-split 3.4MB + bank fp16 4.7MB + out bf16
6.4MB at 360GB/s); DVE ~55us; ACT ~30us; GPSIMD ~33us.
"""

import contextlib
import sys

sys.path.insert(0, "/opt/trn_rl_repo")

import numpy as np
import ml_dtypes

import concourse.bass as bass  # noqa: F401
import concourse.bass_isa as bass_isa
import concourse.mybir as mybir
from concourse.ap import AP
from concourse.tile import TileContext
from concourse import bass_utils, bacc

F32 = mybir.dt.float32
F16 = mybir.dt.float16
BF16 = mybir.dt.bfloat16
FP8 = mybir.dt.float8e4
DR = mybir.MatmulPerfMode.DoubleRow

B, C, H, W = 32, 128, 56, 56
OUT_C, KH, KW = 256, 3, 3
E = 8
N_CORES = 8
BPC = B // N_CORES          # samples per core
HP = H + 2                  # rows incl zero pad
WP = W + 2                  # cols incl zero pad
REG = HP * WP               # 3364 padded pixels per split half
XH0, XL0 = 1, REG + 2       # region bases inside the packed tile
XT = 2 * REG + 3            # 6731: [z | xh | z | xl | z]
NPIX = H * W                # 3136 output pixels
TAPS = KH * KW              # 9 kernel taps, row-major (dy, dx)
RB = 8                      # output rows per PSUM block
NBLK = H // RB              # 7 row blocks
NB = RB * WP                # 464 = matmul N (pad cols dropped at drain)
OCC = OUT_C // 128          # 2 output-channel chunks
PK = TAPS * 128             # 1152 weight cols per (oc, e)
OBUF = NBLK * RB * W        # 3136 drained cols per (oc, sample)

SW = float(2.0 ** 10)       # weight scale before fp8 split
SX = float(2.0 ** 5)        # x scale before fp8 split
SINV = float(2.0 ** -15)    # descale folded into the drain

PAIRS = [(0, 1), (2, 3), (4, 5), (6, 7)]
CH4 = [(0, 2), (2, 4), (4, 7), (7, 9)]          # oc1 startup chunks
CH5 = [(0, 1), (1, 3), (3, 5), (5, 7), (7, 9)]  # oc0 startup chunks

_CACHED_NC = None


def _tap_off(t, r0):
    """Window start of tap t for a block at output row r0, relative to a
    region base (window = contiguous [8 rows x 58 cols] incl wrap)."""
    dy, dx = divmod(t, KW)
    return (r0 + dy) * WP + dx - 1


def _plan_chunks(chunks):
    """Per-chunk DR matmul lists: each A-tap with its chunk, each B pair
    (and the final C) in the first chunk whose taps cover it."""
    plan = []
    for (t0, t1) in chunks:
        kinds = [("A", t) for t in range(t0, t1)]
        kinds += [("B", p0) for (p0, p1) in PAIRS if t0 <= p1 < t1]
        if t1 == TAPS:
            kinds.append(("C", 8))
        plan.append(kinds)
    return plan


def _build_nc():
    nc = bacc.Bacc("TRN2", target_bir_lowering=False, debug=False,
                   num_devices=N_CORES)

    x_d = nc.dram_tensor("x8", [BPC, C, XT], FP8, kind="ExternalInput").ap()
    w_d = nc.dram_tensor("wbank", [C, OCC, E, PK], F16,
                         kind="ExternalInput").ap()
    # fused consts: [:, 0:8]=rwT, [:, 8:16]=rbias, [:, 16:32]=biasT
    c_d = nc.dram_tensor("consts", [128, 2 * E + OCC * E], F32,
                         kind="ExternalInput").ap()
    out_d = nc.dram_tensor("out", [BPC, OCC, 128, OBUF], BF16,
                           kind="ExternalOutput").ap()

    with TileContext(nc) as tc:
        with (
            tc.tile_pool(name="const", bufs=1) as cpool,
            tc.tile_pool(name="xin", bufs=3) as xpool,
            tc.tile_pool(name="mgen", bufs=3) as mpool,
            tc.tile_pool(name="wspl", bufs=4) as wpool,
            tc.tile_pool(name="outp", bufs=3) as opool,
            tc.tile_pool(name="small", bufs=2) as spool,
            tc.tile_pool(name="conv_ps", bufs=NBLK, space="PSUM") as pspool,
            tc.tile_pool(name="warm_ps", bufs=1, space="PSUM") as wpps,
        ):
            # hoist the Sigmoid activation-table load to program start
            # (the sigmoid table also serves Copy/Identity, so this is
            # the only table load in the program)
            dummy = cpool.tile([128, 1], F32, tag="dummy")
            nc.vector.memset(dummy[:, :], 0.0)
            nc.scalar.activation(dummy[:, :], dummy[:, :],
                                 mybir.ActivationFunctionType.Sigmoid)

            # PE warm-up: dependency-free matmuls absorb the pstate ramp
            # during the data-dependent startup
            warm = cpool.tile([128, 512], BF16, tag="warm")
            nc.vector.memset(warm[:, :], 0.0)
            ones = cpool.tile([C, 128], F32, tag="ones")
            nc.vector.memset(ones[:, :], 1.0)
            wps_t = wpps.tile([128, 512], F32, tag="wps", name="warm_psum")
            for _wi in range(11):
                nc.tensor.matmul(wps_t[:, :], lhsT=warm[:, :128],
                                 rhs=warm[:, :], start=True, stop=True)

            def warm_tail(n):
                # disjoint from the logits slice wps_t[:, 0:E]
                for _wi in range(n):
                    nc.tensor.matmul(wps_t[:, 128:256], lhsT=warm[:, :128],
                                     rhs=warm[:, :128], start=True,
                                     stop=True)

            consts = cpool.tile([128, 2 * E + OCC * E], F32, tag="consts")
            rwT = consts[:, 0:E]
            rbb = consts[:, E:2 * E]
            biasT_v = consts[:, 2 * E:].rearrange("c (o e) -> c o e", e=E)

            bank = cpool.tile([C, OCC * E * PK], F16, tag="bank")
            bank_v = bank[:, :].rearrange("c (o e k) -> c o e k", o=OCC, e=E)

            def bank_dma(oc, t0, t1):
                nc.sync.dma_start(out=bank_v[:, oc, :, t0 * 128:t1 * 128],
                                  in_=w_d[:, oc, :, t0 * 128:t1 * 128])

            x_tiles = {}

            def x_dma_xh(b, thirds=False):
                x_tiles[b] = xpool.tile([C, XT], FP8, name=f"xt_{b}",
                                        tag="xt")
                cuts = (0, 1123, 2245, 3366) if thirds else (0, 3366)
                for lo, hi in zip(cuts[:-1], cuts[1:]):
                    nc.sync.dma_start(out=x_tiles[b][:, lo:hi],
                                      in_=x_d[b][:, lo:hi])

            def x_dma_xl(b):
                nc.sync.dma_start(out=x_tiles[b][:, 3366:],
                                  in_=x_d[b][:, 3366:])

            def routing(b, hp):
                """Pool the xh half, sigmoid the logits, form the
                per-sample output bias.  Returns (r_bc, bb)."""
                xt = x_tiles[b]
                with hp:
                    ph = spool.tile([C, 3], F32, tag="ph", name=f"ph_{b}")
                    scr = spool.tile([C, REG // 2], FP8, tag="scr",
                                     name=f"scr_{b}")
                    HH = REG // 2
                    if b == 0:
                        # startup: pool in thirds, middle one on DVE, so
                        # the routing chain launches right as the last x
                        # chunk lands
                        T3 = 1122
                        nc.scalar.activation(
                            scr[:, :T3], xt[:, XH0:XH0 + T3],
                            mybir.ActivationFunctionType.Copy,
                            accum_out=ph[:, 0:1])
                        nc.vector.reduce_sum(
                            out=ph[:, 1:2],
                            in_=xt[:, XH0 + T3:XH0 + 2 * T3],
                            axis=mybir.AxisListType.X)
                        nc.scalar.activation(
                            scr[:, :REG - 2 * T3],
                            xt[:, XH0 + 2 * T3:XH0 + REG],
                            mybir.ActivationFunctionType.Copy,
                            accum_out=ph[:, 2:3])
                        psrc = ph[:, :]
                    else:
                        nc.scalar.activation(
                            scr[:, :], xt[:, XH0:XH0 + HH],
                            mybir.ActivationFunctionType.Copy,
                            accum_out=ph[:, 0:1])
                        nc.scalar.activation(
                            scr[:, :], xt[:, XH0 + HH:XH0 + REG],
                            mybir.ActivationFunctionType.Copy,
                            accum_out=ph[:, 1:2])
                        psrc = ph[:, 0:2]
                    pooled = spool.tile([C, 1], F32, tag="pooled",
                                        name=f"pooled_{b}")
                    nc.vector.reduce_sum(out=pooled[:, :], in_=psrc,
                                         axis=mybir.AxisListType.X)
                    # rp = rwT*pooled + rbb/C; summing over partitions
                    # then yields logits + rbb directly
                    rp = spool.tile([C, E], F32, tag="rp", name=f"rp_{b}")
                    nc.vector.scalar_tensor_tensor(
                        out=rp[:, :], in0=rwT, scalar=pooled[:, :],
                        in1=rbb, op0=mybir.AluOpType.mult,
                        op1=mybir.AluOpType.add)
                    r_bc = spool.tile([C, E], F32, tag="rbc",
                                      name=f"rbc_{b}")
                    if b == 0:
                        # partition-sum via a tiny PE matmul against ones
                        # (the PE is idle during startup; GPSIMD's
                        # all-reduce launch latency sits on the critical
                        # path here)
                        lg_ps = wps_t[:, 0:E]
                        nc.tensor.matmul(lg_ps, lhsT=ones[:, :],
                                         rhs=rp[:, :], start=True,
                                         stop=True)
                        nc.scalar.activation(
                            r_bc[:, :], lg_ps,
                            mybir.ActivationFunctionType.Sigmoid)
                    else:
                        nc.gpsimd.partition_all_reduce(
                            rp[:, :], rp[:, :], C, bass_isa.ReduceOp.add)
                        nc.scalar.activation(
                            r_bc[:, :], rp[:, :],
                            mybir.ActivationFunctionType.Sigmoid)

                    bbt = spool.tile([C, OCC * E], F32, tag="bbt",
                                     name=f"bbt_{b}")
                    bbt_v = bbt[:, :].rearrange("c (o e) -> c o e", e=E)
                    for oc in range(OCC):
                        nc.vector.tensor_mul(out=bbt_v[:, oc, :],
                                             in0=biasT_v[:, oc, :],
                                             in1=r_bc[:, :])
                    bb = spool.tile([128, OCC], F32, tag="bb",
                                    name=f"bb_{b}")
                    nc.vector.reduce_sum(out=bb[:, :], in_=bbt_v[:, :, :],
                                         axis=mybir.AxisListType.X)
                return r_bc, bb

            wstate = {}   # (b, oc) -> (mt_v, wt, wt_v)

            def wgen_alloc(b, oc):
                mt = mpool.tile([C, E * PK], F16, name=f"mt_{b}_{oc}",
                                tag="mt")
                wt = wpool.tile([C, 2 * PK], FP8, name=f"wt_{b}_{oc}",
                                tag="wt")
                wstate[(b, oc)] = (
                    mt[:, :].rearrange("c (e k) -> c e k", e=E),
                    wt,
                    wt[:, :].rearrange("c (s k) -> c s k", s=2),
                )
                return wstate[(b, oc)]

            def wgen_chunk(b, oc, r_bc, t0, t1, hp=None, fast_split=False,
                           act_cast=False):
                """Split weights for taps [t0, t1): DVE mul/add tree in
                fp16, ACT hi-cast, then lo-subtract (DVE for the
                startup-critical chunks, GPSIMD otherwise)."""
                mt_v, _wt, wt_v = wstate[(b, oc)]
                c0, c1 = t0 * 128, t1 * 128
                with hp or contextlib.nullcontext():
                    for e in range(E):
                        nc.vector.tensor_scalar_mul(
                            out=mt_v[:, e, c0:c1],
                            in0=bank_v[:, oc, e, c0:c1],
                            scalar1=r_bc[:, e:e + 1])
                    nc.vector.tensor_add(out=mt_v[:, 0:4, c0:c1],
                                         in0=mt_v[:, 0:4, c0:c1],
                                         in1=mt_v[:, 4:8, c0:c1])
                    nc.vector.tensor_add(out=mt_v[:, 0:2, c0:c1],
                                         in0=mt_v[:, 0:2, c0:c1],
                                         in1=mt_v[:, 2:4, c0:c1])
                    nc.vector.tensor_add(out=mt_v[:, 0, c0:c1],
                                         in0=mt_v[:, 0, c0:c1],
                                         in1=mt_v[:, 1, c0:c1])
                    if fast_split or act_cast:
                        nc.scalar.activation(
                            wt_v[:, 0, c0:c1], mt_v[:, 0, c0:c1],
                            mybir.ActivationFunctionType.Copy)
                    else:
                        nc.gpsimd.tensor_copy(out=wt_v[:, 0, c0:c1],
                                              in_=mt_v[:, 0, c0:c1])
                    if fast_split:
                        nc.vector.tensor_sub(out=wt_v[:, 1, c0:c1],
                                             in0=mt_v[:, 0, c0:c1],
                                             in1=wt_v[:, 0, c0:c1])
                    else:
                        nc.gpsimd.tensor_sub(out=wt_v[:, 1, c0:c1],
                                             in0=mt_v[:, 0, c0:c1],
                                             in1=wt_v[:, 0, c0:c1])

            def dr_mm(ps, kind, t, blk, xt, wt_v, start, stop):
                r0 = blk * RB
                xap = xt[:, :]
                if kind == "A":        # [wh_t, wl_t] x [xh_t, xh_t]
                    lhsT = wt_v[:, :, t * 128:(t + 1) * 128]
                    base = XH0 + _tap_off(t, r0)
                    rhs = xap[:, base:base + NB].unsqueeze(1) \
                        .broadcast_to([C, 2, NB])
                elif kind == "B":      # [wh_t, wh_t1] x [xl_t, xl_t1]
                    hi = wt_v[:, 0, t * 128:(t + 2) * 128]
                    lhsT = hi.rearrange("c (s k) -> c s k", s=2)
                    o0 = _tap_off(t, r0)
                    d = _tap_off(t + 1, r0) - o0
                    sl = xap[:, XL0 + o0:XL0 + o0 + NB]
                    rhs = AP(sl.tensor, sl.offset,
                             [list(sl.ap[0]), [d, 2], [1, NB]])
                else:                  # C: [wh_t, wl_t] x [xl_t, xl_t]
                    lhsT = wt_v[:, :, t * 128:(t + 1) * 128]
                    base = XL0 + _tap_off(t, r0)
                    rhs = xap[:, base:base + NB].unsqueeze(1) \
                        .broadcast_to([C, 2, NB])
                nc.tensor.matmul(ps[:, :], lhsT=lhsT, rhs=rhs,
                                 start=start, stop=stop, perf_mode=DR)

            def drain(ps, oc, blk, obuf, bb):
                psv = ps[:, :].rearrange("m (r w) -> m r w", w=WP)[:, :, 1:57]
                ov = obuf[:, blk * RB * W:(blk + 1) * RB * W] \
                    .rearrange("m (r w) -> m r w", w=W)
                nc.scalar.activation(ov, psv,
                                     mybir.ActivationFunctionType.Identity,
                                     bias=bb[:, oc:oc + 1], scale=SINV)

            def out_dma(b, oc, obuf, cuts):
                for lo, hi in zip(cuts[:-1], cuts[1:]):
                    lo *= RB * W
                    hi *= RB * W
                    nc.sync.dma_start(out=out_d[b, oc, :, lo:hi],
                                      in_=obuf[:, lo:hi])

            def conv_group_chunked(b, oc, bb, chunks):
                """Tap-major conv following the weight chunks (startup
                groups): PE starts on chunk 1 while later chunks load.
                The final chunk runs block-sequential with its drain so
                PSUM banks release early for the next group."""
                xt = x_tiles[b]
                _mt, _wt, wt_v = wstate[(b, oc)]
                obuf = opool.tile([128, OBUF], BF16, name=f"ob_{b}_{oc}",
                                  tag="ob")
                ps_tiles = [pspool.tile([128, NB], F32, tag="cps",
                                        name=f"cps_{b}_{oc}_{blk}")
                            for blk in range(NBLK)]
                nmm = [0] * NBLK
                plan = _plan_chunks(chunks)
                for kinds in plan[:-1]:
                    for kind, t in kinds:
                        for blk in range(NBLK):
                            dr_mm(ps_tiles[blk], kind, t, blk, xt, wt_v,
                                  start=(nmm[blk] == 0),
                                  stop=(nmm[blk] == 13))
                            nmm[blk] += 1
                for blk in range(NBLK):
                    for kind, t in plan[-1]:
                        dr_mm(ps_tiles[blk], kind, t, blk, xt, wt_v,
                              start=(nmm[blk] == 0),
                              stop=(nmm[blk] == 13))
                        nmm[blk] += 1
                    drain(ps_tiles[blk], oc, blk, obuf, bb)
                    if blk == 3:
                        out_dma(b, oc, obuf, (0, 4))
                out_dma(b, oc, obuf, (4, NBLK))

            def conv_group(b, oc, bb, last=False):
                """Block-major conv: each block's 14 matmuls then its
                drain, so PSUM banks recycle smoothly."""
                xt = x_tiles[b]
                _mt, _wt, wt_v = wstate[(b, oc)]
                obuf = opool.tile([128, OBUF], BF16, name=f"ob_{b}_{oc}",
                                  tag="ob")
                kinds = _plan_chunks([(0, TAPS)])[0]
                cuts = (0, 2, 4, 6, NBLK) if last else (0, 4, NBLK)
                ci = 1
                for blk in range(NBLK):
                    ps = pspool.tile([128, NB], F32, tag="cps",
                                     name=f"cps_{b}_{oc}_{blk}")
                    for i, (kind, t) in enumerate(kinds):
                        dr_mm(ps, kind, t, blk, xt, wt_v,
                              start=(i == 0), stop=(i == 13))
                    drain(ps, oc, blk, obuf, bb)
                    if ci < len(cuts) - 1 and blk == cuts[ci] - 1:
                        out_dma(b, oc, obuf, cuts[ci - 1:ci + 1])
                        ci += 1
                out_dma(b, oc, obuf, cuts[-2:])

            # ================= emission schedule =======================
            # prologue: sample 0 startup chain with the bank DMAs chasing
            # the weight-gen chunks, then sample 1 prep
            x_dma_xh(0, thirds=True)
            r0_bc, bb0 = routing(0, tc.high_priority())
            warm_tail(26)
            nc.sync.dma_start(out=consts[:, :], in_=c_d[:, :])
            bank_dma(0, *CH5[0])
            bank_dma(0, *CH5[1])
            wgen_alloc(0, 0)
            wgen_chunk(0, 0, r0_bc, *CH5[0], hp=tc.high_priority(),
                       fast_split=True)
            wgen_chunk(0, 0, r0_bc, *CH5[1], fast_split=True)
            bank_dma(0, *CH5[2])
            x_dma_xl(0)
            wgen_chunk(0, 0, r0_bc, *CH5[2])
            bank_dma(0, *CH5[3])
            wgen_chunk(0, 0, r0_bc, *CH5[3])
            bank_dma(0, *CH5[4])
            wgen_chunk(0, 0, r0_bc, *CH5[4])
            x_dma_xh(1)
            r1_bc, bb1 = routing(1, contextlib.nullcontext())
            wgen_alloc(0, 1)
            bank_dma(1, *CH4[0])
            wgen_chunk(0, 1, r0_bc, *CH4[0], fast_split=True)
            bank_dma(1, *CH4[1])
            wgen_chunk(0, 1, r0_bc, *CH4[1], fast_split=True)
            bank_dma(1, *CH4[2])
            wgen_chunk(0, 1, r0_bc, *CH4[2])
            bank_dma(1, *CH4[3])
            x_dma_xl(1)
            wgen_chunk(0, 1, r0_bc, *CH4[3])
            wgen_alloc(1, 0)
            wgen_chunk(1, 0, r1_bc, 0, 5)
            wgen_chunk(1, 0, r1_bc, 5, TAPS)
            wgen_alloc(1, 1)
            wgen_chunk(1, 1, r1_bc, 0, TAPS)

            conv_group_chunked(0, 0, bb0, CH5)
            conv_group_chunked(0, 1, bb0, CH4)

            rbcs = {1: (r1_bc, bb1)}
            for b in range(1, BPC):
                if b + 1 < BPC:
                    x_dma_xh(b + 1)
                    x_dma_xl(b + 1)
                r_bc, bb = rbcs[b]
                conv_group(b, 0, bb)
                if b > 1:
                    wgen_alloc(b, 1)
                    wgen_chunk(b, 1, r_bc, 0, TAPS)
                if b + 1 < BPC:
                    rbcs[b + 1] = routing(b + 1, contextlib.nullcontext())
                conv_group(b, 1, bb, last=(b == BPC - 1))
                if b + 1 < BPC:
                    wgen_alloc(b + 1, 0)
                    wgen_chunk(b + 1, 0, rbcs[b + 1][0], 0, TAPS)

    nc.compile()
    return nc


def _get_nc():
    global _CACHED_NC
    if _CACHED_NC is None:
        _CACHED_NC = _build_nc()
    return _CACHED_NC


def _prepare_in_maps(x, weight, routing_weight, routing_bias, bias):
    f8 = ml_dtypes.float8_e4m3

    xs = np.zeros((B, C, HP, WP), dtype=np.float32)
    xs[:, :, 1:1 + H, 1:1 + W] = x * SX
    xh = xs.astype(f8)
    xl = (xs - xh.astype(np.float32)).astype(f8)
    xp = np.zeros((B, C, XT), dtype=f8)
    xp[:, :, XH0:XH0 + REG] = xh.reshape(B, C, REG)
    xp[:, :, XL0:XL0 + REG] = xl.reshape(B, C, REG)

    # bank[c, oc, e, t*128 + m] = SW * weight[e, (oc*128+m, c, tap t)]
    wr = (weight * SW).reshape(E, OCC, 128, C, TAPS).transpose(3, 1, 0, 4, 2)
    wr = np.ascontiguousarray(wr).astype(np.float16).reshape(C, OCC, E, PK)

    consts = np.zeros((128, 2 * E + OCC * E), dtype=np.float32)
    consts[:, 0:E] = routing_weight.T / (NPIX * SX)
    consts[:, E:2 * E] = routing_bias.reshape(1, E) / C
    consts[:, 2 * E:] = bias.T.reshape(OCC, 128, E).transpose(1, 0, 2) \
        .reshape(128, OCC * E)

    in_maps = []
    for c in range(N_CORES):
        in_maps.append({
            "x8": np.ascontiguousarray(xp[c * BPC:(c + 1) * BPC]),
            "wbank": wr,
            "consts": consts,
        })
    return in_maps


def kernel(x, weight, routing_weight, routing_bias, bias, _trace=False):
    nc = _get_nc()
    in_maps = _prepare_in_maps(np.asarray(x, dtype=np.float32),
                               np.asarray(weight, dtype=np.float32),
                               np.asarray(routing_weight, dtype=np.float32),
                               np.asarray(routing_bias, dtype=np.float32),
                               np.asarray(bias, dtype=np.float32))
    res = bass_utils.run_bass_kernel_spmd(
        nc, in_maps, core_ids=list(range(N_CORES)), trace=_trace)
    out = np.concatenate([np.asarray(res.results[c]["out"])
                          for c in range(N_CORES)], axis=0)
    out = out.astype(np.float32).reshape(B, OUT_C, H, W)
    if _trace:
        kernel.last_results = res
    return out
